# revision 14
# baseline (speedup 1.0000x reference)
"""Trainium2 Bass kernel for nn_AdaptiveDecoder (shared MLP + hard-routed type heads).

Strategy:
  * Host: sort nodes by type; pad each type's count to a multiple of 8*128 and
    split evenly over 8 cores -> every core sees the SAME static layout of
    type-pure 128-row tiles, so the compiled SPMD program bakes in the
    tile->head mapping and the device does zero routing work.
  * Device: keep activations transposed ([feature, nodes]) the whole way so
    the three matmul stages chain without transposes.  LayerNorm's gamma/beta
    are folded into the head weights on the host; the per-node mean/std terms
    enter via K=1 rank-1 accumulation matmuls and one K=1 broadcast matmul.
  * Matmuls run as float32r (full-rate fp32 path on TRN2 TensorE).
"""

import sys

sys.path.insert(0, "/opt/trn_rl_repo")

from contextlib import ExitStack

import numpy as np

N_CORES = 8
LATENT, HIDDEN, OUT, TYPES = 512, 1024, 256, 3
P = 128
NB = 512  # node columns per block (moving-dim max for 4-byte matmul)
KL = LATENT // P  # 4 k-tiles, stage 1
KH = HIDDEN // P  # 8 k-tiles, stage 2 / head
MH = HIDDEN // P  # 8 m-chunks of hidden
MO = OUT // P  # 2 m-chunks of head output
LN_EPS = 1e-5


def build_program(blocks, R):
    """blocks: list of (type_idx, col_offset, n_cols); R: node columns per core."""
    import concourse.mybir as mybir
    import concourse.tile as tile
    from concourse import bacc

    dt = mybir.dt
    f32, f32r, bf16 = dt.float32, dt.float32r, dt.bfloat16
    AF = mybir.ActivationFunctionType

    nc = bacc.Bacc("TRN2", target_bir_lowering=False, debug=False, num_devices=N_CORES)

    xt = nc.dram_tensor("xt", [LATENT, R], f32r, kind="ExternalInput").ap()
    w1d = nc.dram_tensor("w1", [LATENT, HIDDEN], f32r, kind="ExternalInput").ap()
    w2d = nc.dram_tensor("w2", [HIDDEN, HIDDEN], f32r, kind="ExternalInput").ap()
    b1d = nc.dram_tensor("b1r", [P, MH], f32, kind="ExternalInput").ap()
    b2d = nc.dram_tensor("b2r", [P, MH], f32, kind="ExternalInput").ap()
    whpd = nc.dram_tensor("whp", [TYPES, HIDDEN, OUT], f32r, kind="ExternalInput").ap()
    c1d = nc.dram_tensor("c1", [TYPES, 1, OUT], f32r, kind="ExternalInput").ap()
    c2d = nc.dram_tensor("c2", [TYPES, 1, OUT], f32r, kind="ExternalInput").ap()
    ocold = nc.dram_tensor("ocol", [P, 1], f32r, kind="ExternalInput").ap()
    orowd = nc.dram_tensor("orow", [1, P], f32r, kind="ExternalInput").ap()
    outd = nc.dram_tensor("out", [OUT, R], f32, kind="ExternalOutput").ap()

    def r(ap):  # operands already declared float32r
        return ap

    with tile.TileContext(nc) as tc, ExitStack() as ctx:
        consts = ctx.enter_context(tc.tile_pool(name="consts", bufs=1))
        xt_pool = ctx.enter_context(tc.tile_pool(name="xt", bufs=2))
        h1_pool = ctx.enter_context(tc.tile_pool(name="h1", bufs=2))
        h2_pool = ctx.enter_context(tc.tile_pool(name="h2", bufs=2))
        sq_pool = ctx.enter_context(tc.tile_pool(name="sq", bufs=2))
        rv_pool = ctx.enter_context(tc.tile_pool(name="rv", bufs=1))
        ab_pool = ctx.enter_context(tc.tile_pool(name="ab", bufs=2))
        out_pool = ctx.enter_context(tc.tile_pool(name="outp", bufs=2))
        ps_mlp = ctx.enter_context(tc.tile_pool(name="ps_mlp", bufs=2, space="PSUM"))
        ps_head = ctx.enter_context(tc.tile_pool(name="ps_head", bufs=2, space="PSUM"))
        ps_stat = ctx.enter_context(tc.tile_pool(name="ps_stat", bufs=2, space="PSUM"))
        ps_bc = ctx.enter_context(tc.tile_pool(name="ps_bc", bufs=1, space="PSUM"))

        # --- constants / weights, loaded once ---
        w1_sb = consts.tile([P, KL * HIDDEN], f32r)
        for k in range(KL):
            nc.sync.dma_start(
                out=w1_sb[:, k * HIDDEN : (k + 1) * HIDDEN],
                in_=w1d[k * P : (k + 1) * P, :],
            )
        w2_sb = consts.tile([P, KH * HIDDEN], f32r)
        for k in range(KH):
            nc.sync.dma_start(
                out=w2_sb[:, k * HIDDEN : (k + 1) * HIDDEN],
                in_=w2d[k * P : (k + 1) * P, :],
            )
        whp_sb = consts.tile([P, TYPES * KH * OUT], f32r)
        for t in range(TYPES):
            for k in range(KH):
                nc.sync.dma_start(
                    out=whp_sb[:, (t * KH + k) * OUT : (t * KH + k + 1) * OUT],
                    in_=whpd[t, k * P : (k + 1) * P, :],
                )
        b1_sb = consts.tile([P, MH], f32)
        nc.sync.dma_start(out=b1_sb[:], in_=b1d[:])
        b2_sb = consts.tile([P, MH], f32)
        nc.sync.dma_start(out=b2_sb[:], in_=b2d[:])
        c1_sb = consts.tile([1, TYPES * OUT], f32r)
        c2_sb = consts.tile([1, TYPES * OUT], f32r)
        for t in range(TYPES):
            nc.sync.dma_start(out=c1_sb[:, t * OUT : (t + 1) * OUT], in_=c1d[t])
            nc.sync.dma_start(out=c2_sb[:, t * OUT : (t + 1) * OUT], in_=c2d[t])
        ones_col = consts.tile([P, 1], f32r)  # lhsT for partition-sum reduce
        nc.sync.dma_start(out=ones_col[:], in_=ocold[:])
        ones_col_bf = consts.tile([P, 1], bf16)
        nc.vector.memset(ones_col_bf[:], 1.0)
        ones_row = consts.tile([1, P], f32r)  # lhsT for partition broadcast
        nc.sync.dma_start(out=ones_row[:], in_=orowd[:])
        eps_ap = consts.tile([1, 1], f32)
        nc.vector.memset(eps_ap[:], LN_EPS)

        # --- per-block pipeline ---
        for t, c0, nb in blocks:
            xt_t = xt_pool.tile([P, KL * NB], f32r, tag="xt")
            for k in range(KL):
                nc.sync.dma_start(
                    out=xt_t[:, k * NB : k * NB + nb],
                    in_=xt[k * P : (k + 1) * P, c0 : c0 + nb],
                )

            # stage 1: h1^T = relu(W1^T x + b1)   [HIDDEN, nb]
            h1_t = h1_pool.tile([P, MH * NB], f32r, tag="h1")
            for m in range(MH):
                ps = ps_mlp.tile([P, NB], f32, tag="ps_mlp")
                for k in range(KL):
                    nc.tensor.matmul(
                        ps[:, :nb],
                        lhsT=r(w1_sb[:, k * HIDDEN + m * P : k * HIDDEN + (m + 1) * P]),
                        rhs=r(xt_t[:, k * NB : k * NB + nb]),
                        start=(k == 0),
                        stop=(k == KL - 1),
                    )
                nc.scalar.activation(
                    h1_t[:, m * NB : m * NB + nb],
                    ps[:, :nb],
                    AF.Relu,
                    bias=b1_sb[:, m : m + 1],
                )

            # stage 2: h2^T = W2^T h1 + b2   [HIDDEN, nb]
            h2_t = h2_pool.tile([P, MH * NB], f32r, tag="h2")
            for m in range(MH):
                ps = ps_mlp.tile([P, NB], f32, tag="ps_mlp")
                for k in range(KH):
                    nc.tensor.matmul(
                        ps[:, :nb],
                        lhsT=r(w2_sb[:, k * HIDDEN + m * P : k * HIDDEN + (m + 1) * P]),
                        rhs=r(h1_t[:, k * NB : k * NB + nb]),
                        start=(k == 0),
                        stop=(k == KH - 1),
                    )
                nc.scalar.activation(
                    h2_t[:, m * NB : m * NB + nb],
                    ps[:, :nb],
                    AF.Identity,
                    bias=b2_sb[:, m : m + 1],
                )

            # LN stats: column sums of h2^T and (h2^T)^2 via ones-matmul
            sq_t = sq_pool.tile([P, MH * NB], bf16, tag="sq")
            for m in range(MH):
                nc.vector.tensor_mul(
                    sq_t[:, m * NB : m * NB + nb],
                    h2_t[:, m * NB : m * NB + nb].bitcast(f32),
                    h2_t[:, m * NB : m * NB + nb].bitcast(f32),
                )
            ps_s = ps_stat.tile([1, NB], f32, tag="stat")
            for k in range(KH):
                nc.tensor.matmul(
                    ps_s[:, :nb],
                    lhsT=r(ones_col[:]),
                    rhs=r(h2_t[:, k * NB : k * NB + nb]),
                    start=(k == 0),
                    stop=(k == KH - 1),
                )
            ps_q = ps_stat.tile([1, NB], f32, tag="stat")
            for k in range(KH):
                nc.tensor.matmul(
                    ps_q[:, :nb],
                    lhsT=ones_col_bf[:],
                    rhs=sq_t[:, k * NB : k * NB + nb],
                    start=(k == 0),
                    stop=(k == KH - 1),
                )

            negmu = rv_pool.tile([1, NB], f32r, tag="negmu")
            nc.scalar.activation(
                negmu[:, :nb], ps_s[:, :nb], AF.Identity, scale=-1.0 / HIDDEN
            )
            musq = rv_pool.tile([1, NB], f32, tag="musq")
            nc.scalar.activation(
                musq[:, :nb], ps_s[:, :nb], AF.Square, scale=1.0 / HIDDEN
            )
            varv = rv_pool.tile([1, NB], f32, tag="varv")
            nc.scalar.activation(
                varv[:, :nb], ps_q[:, :nb], AF.Identity, scale=1.0 / HIDDEN
            )
            nc.vector.tensor_sub(varv[:, :nb], varv[:, :nb], musq[:, :nb])
            sv = rv_pool.tile([1, NB], f32r, tag="sv")  # sqrt(var + eps)
            nc.scalar.activation(sv[:, :nb], varv[:, :nb], AF.Sqrt, bias=eps_ap[:])
            rsig = rv_pool.tile([1, NB], f32r, tag="rsig")
            with nc.allow_low_precision(reason="tf32 rsig feeds f32r bcast matmul"):
                nc.vector.reciprocal(rsig[:, :nb], sv[:, :nb].bitcast(f32))

            # broadcast rsig across partitions (K=1 matmul), stash in SBUF
            ps_a = ps_bc.tile([P, NB], f32, tag="bc")
            nc.tensor.matmul(
                ps_a[:, :nb],
                lhsT=r(ones_row[:]),
                rhs=r(rsig[:, :nb]),
                start=True,
                stop=True,
            )
            a_sb = ab_pool.tile([P, NB], f32, tag="a")
            nc.scalar.activation(a_sb[:, :nb], ps_a[:, :nb], AF.Identity)

            # head: psum = Wh'^T h2 + (-mu) x c2 + sv x c1 ; out = rsig * psum
            out_sb = out_pool.tile([P, MO * NB], f32, tag="out")
            for mc in range(MO):
                ph = ps_head.tile([P, NB], f32, tag="head")
                for k in range(KH):
                    nc.tensor.matmul(
                        ph[:, :nb],
                        lhsT=r(
                            whp_sb[
                                :,
                                (t * KH + k) * OUT + mc * P : (t * KH + k) * OUT
                                + (mc + 1) * P,
                            ]
                        ),
                        rhs=r(h2_t[:, k * NB : k * NB + nb]),
                        start=(k == 0),
                        stop=False,
                    )
                nc.tensor.matmul(
                    ph[:, :nb],
                    lhsT=r(c2_sb[:, t * OUT + mc * P : t * OUT + (mc + 1) * P]),
                    rhs=r(negmu[:, :nb]),
                    start=False,
                    stop=False,
                )
                nc.tensor.matmul(
                    ph[:, :nb],
                    lhsT=r(c1_sb[:, t * OUT + mc * P : t * OUT + (mc + 1) * P]),
                    rhs=r(sv[:, :nb]),
                    start=False,
                    stop=True,
                )
                nc.vector.tensor_mul(
                    out_sb[:, mc * NB : mc * NB + nb], ph[:, :nb], a_sb[:, :nb]
                )
                nc.sync.dma_start(
                    out=outd[mc * P : (mc + 1) * P, c0 : c0 + nb],
                    in_=out_sb[:, mc * NB : mc * NB + nb],
                )

    nc.compile()
    return nc


def plan(node_types):
    """Host-side layout plan shared by all cores.

    Returns (blocks, R, caps, idx_by_type) where idx_by_type[t][c] is the array
    of original row indices of type t assigned to core c.
    """
    node_types = np.asarray(node_types)
    counts = np.bincount(node_types, minlength=TYPES)
    caps = []  # per-core column capacity for each type (multiple of P)
    idx_by_type = []
    order = np.argsort(node_types, kind="stable")
    starts = np.concatenate([[0], np.cumsum(counts)])
    for tt in range(TYPES):
        cap = int(-(-counts[tt] // (N_CORES * P))) * P  # ceil to 128 per core
        caps.append(cap)
        idx_t = order[starts[tt] : starts[tt + 1]]
        base, rem = divmod(int(counts[tt]), N_CORES)
        parts, o = [], 0
        for c in range(N_CORES):
            n = base + (1 if c < rem else 0)
            parts.append(idx_t[o : o + n])
            o += n
        idx_by_type.append(parts)
    R = sum(caps)
    blocks = []
    off = 0
    for tt in range(TYPES):
        tiles = caps[tt] // P
        j = 0
        while j < tiles:
            nt = min(NB // P, tiles - j)
            blocks.append((tt, off + j * P, nt * P))
            j += nt
        off += caps[tt]
    return blocks, R, caps, idx_by_type


def _tf32(x):
    """Round fp32 to TF32 (10-bit mantissa, round-to-nearest-even)."""
    u = np.ascontiguousarray(x, dtype=np.float32).view(np.uint32).copy()
    lsb = (u >> np.uint32(13)) & np.uint32(1)
    u += np.uint32(0x0FFF) + lsb
    u &= np.uint32(0xFFFFE000)
    return u.view(np.float32)


def prep_inputs(node_latent, w1, b1, w2, b2, ln_gamma, ln_beta, head_w, head_b,
                caps, idx_by_type):
    """Build the 8 per-core input maps."""
    whp = _tf32(ln_gamma[:, None] * head_w)  # [T, H, OUT]
    c1 = _tf32(np.asarray(ln_beta @ head_w + head_b)).reshape(TYPES, 1, OUT)
    c2 = _tf32(np.asarray(ln_gamma @ head_w)).reshape(TYPES, 1, OUT)
    b1r = np.ascontiguousarray(b1.reshape(MH, P).T).astype(np.float32)
    b2r = np.ascontiguousarray(b2.reshape(MH, P).T).astype(np.float32)
    R = sum(caps)
    in_maps = []
    for c in range(N_CORES):
        xc = np.zeros((R, LATENT), np.float32)
        off = 0
        for tt in range(TYPES):
            idx = idx_by_type[tt][c]
            xc[off : off + len(idx)] = node_latent[idx]
            off += caps[tt]
        in_maps.append(
            {
                "xt": _tf32(xc.T),
                "w1": _tf32(w1),
                "w2": _tf32(w2),
                "b1r": b1r,
                "b2r": b2r,
                "whp": whp,
                "c1": c1,
                "c2": c2,
                "ocol": np.ones((P, 1), np.float32),
                "orow": np.ones((1, P), np.float32),
            }
        )
    return in_maps


def unpack_outputs(results, caps, idx_by_type, n_rows):
    out = np.empty((n_rows, OUT), np.float32)
    for c in range(N_CORES):
        oc = results[c]["out"]  # [OUT, R]
        off = 0
        for tt in range(TYPES):
            idx = idx_by_type[tt][c]
            out[idx] = oc[:, off : off + len(idx)].T
            off += caps[tt]
    return out


def kernel(node_latent, node_types, w1, b1, w2, b2, ln_gamma, ln_beta, head_w, head_b):
    from concourse.bass_utils import run_bass_kernel_spmd

    node_latent = np.asarray(node_latent, dtype=np.float32)
    node_types = np.asarray(node_types)
    blocks, R, caps, idx_by_type = plan(node_types)
    nc = build_program(blocks, R)
    in_maps = prep_inputs(
        node_latent, w1, b1, w2, b2, ln_gamma, ln_beta, head_w, head_b,
        caps, idx_by_type,
    )
    res = run_bass_kernel_spmd(nc, in_maps, core_ids=list(range(N_CORES)))
    return unpack_outputs(res.results, caps, idx_by_type, node_latent.shape[0])


# revision 17
# speedup vs baseline: 1.1503x; 1.1503x over previous
"""Trainium2 Bass kernel for nn_AdaptiveDecoder (shared MLP + hard-routed type heads).

Strategy:
  * Host: sort nodes by type; pad each type's count to a multiple of 8*128 and
    split evenly over 8 cores -> every core sees the SAME static layout of
    type-pure 128-row tiles, so the compiled SPMD program bakes in the
    tile->head mapping and the device does zero routing work.
  * Device: keep activations transposed ([feature, nodes]) the whole way so
    the three matmul stages chain without transposes.  LayerNorm's gamma/beta
    are folded into the head weights on the host; the per-node mean/std terms
    enter via K=1 rank-1 accumulation matmuls and one K=1 broadcast matmul.
  * Matmuls run as float32r (full-rate fp32 path on TRN2 TensorE).
"""

import sys

sys.path.insert(0, "/opt/trn_rl_repo")

from contextlib import ExitStack

import numpy as np

N_CORES = 8
LATENT, HIDDEN, OUT, TYPES = 512, 1024, 256, 3
P = 128
NB = 512  # node columns per block (moving-dim max for 4-byte matmul)
KL = LATENT // P  # 4 k-tiles, stage 1
KH = HIDDEN // P  # 8 k-tiles, stage 2 / head
MH = HIDDEN // P  # 8 m-chunks of hidden
MO = OUT // P  # 2 m-chunks of head output
LN_EPS = 1e-5


def build_program(blocks, R, use_c1=True):
    """blocks: list of (type_idx, col_offset, n_cols); R: node columns per core."""
    import concourse.mybir as mybir
    import concourse.tile as tile
    from concourse import bacc

    dt = mybir.dt
    f32, f32r, bf16 = dt.float32, dt.float32r, dt.bfloat16
    AF = mybir.ActivationFunctionType

    nc = bacc.Bacc("TRN2", target_bir_lowering=False, debug=False, num_devices=N_CORES)

    xt = nc.dram_tensor("xt", [LATENT, R], f32r, kind="ExternalInput").ap()
    w1d = nc.dram_tensor("w1", [LATENT, HIDDEN], f32r, kind="ExternalInput").ap()
    w2d = nc.dram_tensor("w2", [HIDDEN, HIDDEN], f32r, kind="ExternalInput").ap()
    b1d = nc.dram_tensor("b1r", [P, MH], f32, kind="ExternalInput").ap()
    b2d = nc.dram_tensor("b2r", [P, MH], f32, kind="ExternalInput").ap()
    whpd = nc.dram_tensor("whp", [TYPES, HIDDEN, OUT], f32r, kind="ExternalInput").ap()
    c1d = nc.dram_tensor("c1", [TYPES, 1, OUT], f32r, kind="ExternalInput").ap()
    c2d = nc.dram_tensor("c2", [TYPES, 1, OUT], f32r, kind="ExternalInput").ap()
    ocold = nc.dram_tensor("ocol", [P, 1], f32r, kind="ExternalInput").ap()
    orowd = nc.dram_tensor("orow", [1, P], f32r, kind="ExternalInput").ap()
    outd = nc.dram_tensor("out", [OUT, R], f32, kind="ExternalOutput").ap()

    def r(ap):  # operands already declared float32r
        return ap

    with tile.TileContext(nc) as tc, ExitStack() as ctx:
        consts = ctx.enter_context(tc.tile_pool(name="consts", bufs=1))
        xt_pool = ctx.enter_context(tc.tile_pool(name="xt", bufs=2))
        h1_pool = ctx.enter_context(tc.tile_pool(name="h1", bufs=2))
        h2_pool = ctx.enter_context(tc.tile_pool(name="h2", bufs=2))
        sq_pool = ctx.enter_context(tc.tile_pool(name="sq", bufs=2))
        rv_pool = ctx.enter_context(tc.tile_pool(name="rv", bufs=1))
        ab_pool = ctx.enter_context(tc.tile_pool(name="ab", bufs=2))
        out_pool = ctx.enter_context(tc.tile_pool(name="outp", bufs=2))
        ps_mlp = ctx.enter_context(tc.tile_pool(name="ps_mlp", bufs=2, space="PSUM"))
        ps_head = ctx.enter_context(tc.tile_pool(name="ps_head", bufs=2, space="PSUM"))
        ps_stat = ctx.enter_context(tc.tile_pool(name="ps_stat", bufs=2, space="PSUM"))
        ps_bc = ctx.enter_context(tc.tile_pool(name="ps_bc", bufs=1, space="PSUM"))

        # --- constants / weights, loaded once ---
        w1_sb = consts.tile([P, KL * HIDDEN], f32r)
        for k in range(KL):
            nc.sync.dma_start(
                out=w1_sb[:, k * HIDDEN : (k + 1) * HIDDEN],
                in_=w1d[k * P : (k + 1) * P, :],
            )
        w2_sb = consts.tile([P, KH * HIDDEN], f32r)
        for k in range(KH):
            nc.sync.dma_start(
                out=w2_sb[:, k * HIDDEN : (k + 1) * HIDDEN],
                in_=w2d[k * P : (k + 1) * P, :],
            )
        whp_sb = consts.tile([P, TYPES * KH * OUT], f32r)
        for t in range(TYPES):
            for k in range(KH):
                nc.sync.dma_start(
                    out=whp_sb[:, (t * KH + k) * OUT : (t * KH + k + 1) * OUT],
                    in_=whpd[t, k * P : (k + 1) * P, :],
                )
        b1_sb = consts.tile([P, MH], f32)
        nc.sync.dma_start(out=b1_sb[:], in_=b1d[:])
        b2_sb = consts.tile([P, MH], f32)
        nc.sync.dma_start(out=b2_sb[:], in_=b2d[:])
        c1_sb = consts.tile([1, TYPES * OUT], f32r)
        c2_sb = consts.tile([1, TYPES * OUT], f32r)
        for t in range(TYPES):
            nc.sync.dma_start(out=c1_sb[:, t * OUT : (t + 1) * OUT], in_=c1d[t])
            nc.sync.dma_start(out=c2_sb[:, t * OUT : (t + 1) * OUT], in_=c2d[t])
        ones_col = consts.tile([P, 1], f32r)  # lhsT for partition-sum reduce
        nc.sync.dma_start(out=ones_col[:], in_=ocold[:])
        ones_col_bf = consts.tile([P, 1], bf16)
        nc.vector.memset(ones_col_bf[:], 1.0)
        ones_row = consts.tile([1, P], f32r)  # lhsT for partition broadcast
        nc.sync.dma_start(out=ones_row[:], in_=orowd[:])
        eps_ap = consts.tile([1, 1], f32)
        nc.vector.memset(eps_ap[:], LN_EPS)

        # --- per-block pipeline (software-pipelined: the LN-dependent PE ops
        # of block b are emitted mid-block b+1 so the PE never waits on the
        # ACT/DVE stats chain and the HAM clock stays warm) ---

        def emit_tail(t, c0, nb, ph_list, negmu, sv, rsig):
            # rank-1 corrections close the head psum accumulation groups
            for mc in range(MO):
                ph = ph_list[mc]
                nc.tensor.matmul(
                    ph[:, :nb],
                    lhsT=c2_sb[:, t * OUT + mc * P : t * OUT + (mc + 1) * P],
                    rhs=negmu[:, :nb],
                    start=False,
                    stop=not use_c1,
                )
                if use_c1:
                    nc.tensor.matmul(
                        ph[:, :nb],
                        lhsT=c1_sb[:, t * OUT + mc * P : t * OUT + (mc + 1) * P],
                        rhs=sv[:, :nb],
                        start=False,
                        stop=True,
                    )
            # broadcast rsig across partitions (K=1 matmul), stash in SBUF
            ps_a = ps_bc.tile([P, NB], f32, tag="bc")
            nc.tensor.matmul(
                ps_a[:, :nb], lhsT=ones_row[:], rhs=rsig[:, :nb],
                start=True, stop=True,
            )
            a_sb = ab_pool.tile([P, NB], f32, tag="a")
            nc.scalar.activation(a_sb[:, :nb], ps_a[:, :nb], AF.Identity)
            out_sb = out_pool.tile([P, MO * NB], f32, tag="out")
            for mc in range(MO):
                nc.vector.tensor_mul(
                    out_sb[:, mc * NB : mc * NB + nb], ph_list[mc][:, :nb],
                    a_sb[:, :nb],
                )
                nc.sync.dma_start(
                    out=outd[mc * P : (mc + 1) * P, c0 : c0 + nb],
                    in_=out_sb[:, mc * NB : mc * NB + nb],
                )

        pending = None
        for t, c0, nb in blocks:
            xt_t = xt_pool.tile([P, KL * NB], f32r, tag="xt")
            for k in range(KL):
                nc.sync.dma_start(
                    out=xt_t[:, k * NB : k * NB + nb],
                    in_=xt[k * P : (k + 1) * P, c0 : c0 + nb],
                )

            # stage 1: h1^T = relu(W1^T x + b1)   [HIDDEN, nb]
            h1_t = h1_pool.tile([P, MH * NB], f32r, tag="h1")
            for m in range(MH):
                ps = ps_mlp.tile([P, NB], f32, tag="ps_mlp")
                for k in range(KL):
                    nc.tensor.matmul(
                        ps[:, :nb],
                        lhsT=w1_sb[:, k * HIDDEN + m * P : k * HIDDEN + (m + 1) * P],
                        rhs=xt_t[:, k * NB : k * NB + nb],
                        start=(k == 0),
                        stop=(k == KL - 1),
                    )
                nc.scalar.activation(
                    h1_t[:, m * NB : m * NB + nb],
                    ps[:, :nb],
                    AF.Relu,
                    bias=b1_sb[:, m : m + 1],
                )

            # deferred LN tail of the previous block slots in here: its PE
            # inputs (negmu/sv/rsig) became ready while stage 1 above ran
            if pending is not None:
                pending()
                pending = None

            # stage 2: h2^T = W2^T h1 + b2; squares ride along per chunk
            h2_t = h2_pool.tile([P, MH * NB], f32r, tag="h2")
            sq_t = sq_pool.tile([P, MH * NB], bf16, tag="sq")
            for m in range(MH):
                ps = ps_mlp.tile([P, NB], f32, tag="ps_mlp")
                for k in range(KH):
                    nc.tensor.matmul(
                        ps[:, :nb],
                        lhsT=w2_sb[:, k * HIDDEN + m * P : k * HIDDEN + (m + 1) * P],
                        rhs=h1_t[:, k * NB : k * NB + nb],
                        start=(k == 0),
                        stop=(k == KH - 1),
                    )
                nc.scalar.activation(
                    h2_t[:, m * NB : m * NB + nb],
                    ps[:, :nb],
                    AF.Identity,
                    bias=b2_sb[:, m : m + 1],
                )
                nc.vector.tensor_mul(
                    sq_t[:, m * NB : m * NB + nb],
                    h2_t[:, m * NB : m * NB + nb].bitcast(f32),
                    h2_t[:, m * NB : m * NB + nb].bitcast(f32),
                )

            # head main matmuls: only need h2, so they keep the PE hot while
            # the stats chain below runs on ACT/DVE
            ph_list = []
            for mc in range(MO):
                ph = ps_head.tile([P, NB], f32, tag="head")
                for k in range(KH):
                    nc.tensor.matmul(
                        ph[:, :nb],
                        lhsT=whp_sb[
                            :,
                            (t * KH + k) * OUT + mc * P : (t * KH + k) * OUT
                            + (mc + 1) * P,
                        ],
                        rhs=h2_t[:, k * NB : k * NB + nb],
                        start=(k == 0),
                        stop=False,
                    )
                ph_list.append(ph)

            # LN stats: column sums of h2^T and (h2^T)^2 via ones-matmul
            ps_s = ps_stat.tile([1, NB], f32, tag="stat")
            for k in range(KH):
                nc.tensor.matmul(
                    ps_s[:, :nb],
                    lhsT=ones_col[:],
                    rhs=h2_t[:, k * NB : k * NB + nb],
                    start=(k == 0),
                    stop=(k == KH - 1),
                )
            ps_q = ps_stat.tile([1, NB], f32, tag="stat")
            for k in range(KH):
                nc.tensor.matmul(
                    ps_q[:, :nb],
                    lhsT=ones_col_bf[:],
                    rhs=sq_t[:, k * NB : k * NB + nb],
                    start=(k == 0),
                    stop=(k == KH - 1),
                )

            negmu = rv_pool.tile([1, NB], f32r, tag="negmu")
            nc.scalar.activation(
                negmu[:, :nb], ps_s[:, :nb], AF.Identity, scale=-1.0 / HIDDEN
            )
            musq = rv_pool.tile([1, NB], f32, tag="musq")
            nc.scalar.activation(
                musq[:, :nb], ps_s[:, :nb], AF.Square, scale=1.0 / HIDDEN
            )
            varv = rv_pool.tile([1, NB], f32, tag="varv")
            nc.scalar.activation(
                varv[:, :nb], ps_q[:, :nb], AF.Identity, scale=1.0 / HIDDEN
            )
            nc.vector.tensor_sub(varv[:, :nb], varv[:, :nb], musq[:, :nb])
            sv = rv_pool.tile([1, NB], f32r, tag="sv")  # sqrt(var + eps)
            nc.scalar.activation(sv[:, :nb], varv[:, :nb], AF.Sqrt, bias=eps_ap[:])
            rsig = rv_pool.tile([1, NB], f32r, tag="rsig")
            with nc.allow_low_precision(reason="tf32 rsig feeds f32r bcast matmul"):
                nc.vector.reciprocal(rsig[:, :nb], sv[:, :nb].bitcast(f32))

            import functools

            pending = functools.partial(
                emit_tail, t, c0, nb, ph_list, negmu, sv, rsig
            )

        pending()

    nc.compile()
    return nc


def plan(node_types):
    """Host-side layout plan shared by all cores.

    Returns (blocks, R, caps, idx_by_type) where idx_by_type[t][c] is the array
    of original row indices of type t assigned to core c.
    """
    node_types = np.asarray(node_types)
    counts = np.bincount(node_types, minlength=TYPES)
    caps = []  # per-core column capacity for each type (multiple of P)
    idx_by_type = []
    order = np.argsort(node_types, kind="stable")
    starts = np.concatenate([[0], np.cumsum(counts)])
    for tt in range(TYPES):
        tiles = int(-(-counts[tt] // (N_CORES * P)))  # ceil to 128-row tiles/core
        if tiles % 4 == 1:
            # a lone 128-col block runs f32r at 1/4 rate - same cost as 2 cols
            tiles += 1
        cap = tiles * P
        caps.append(cap)
        idx_t = order[starts[tt] : starts[tt + 1]]
        base, rem = divmod(int(counts[tt]), N_CORES)
        parts, o = [], 0
        for c in range(N_CORES):
            n = base + (1 if c < rem else 0)
            parts.append(idx_t[o : o + n])
            o += n
        idx_by_type.append(parts)
    R = sum(caps)
    blocks = []
    off = 0
    for tt in range(TYPES):
        tiles = caps[tt] // P
        j = 0
        while j < tiles:
            nt = min(NB // P, tiles - j)
            blocks.append((tt, off + j * P, nt * P))
            j += nt
        off += caps[tt]
    return blocks, R, caps, idx_by_type


def _tf32(x):
    """Round fp32 to TF32 (10-bit mantissa, round-to-nearest-even)."""
    u = np.ascontiguousarray(x, dtype=np.float32).view(np.uint32).copy()
    lsb = (u >> np.uint32(13)) & np.uint32(1)
    u += np.uint32(0x0FFF) + lsb
    u &= np.uint32(0xFFFFE000)
    return u.view(np.float32)


def prep_inputs(node_latent, w1, b1, w2, b2, ln_gamma, ln_beta, head_w, head_b,
                caps, idx_by_type):
    """Build the 8 per-core input maps."""
    whp = _tf32(ln_gamma[:, None] * head_w)  # [T, H, OUT]
    c1 = _tf32(np.asarray(ln_beta @ head_w + head_b)).reshape(TYPES, 1, OUT)
    c2 = _tf32(np.asarray(ln_gamma @ head_w)).reshape(TYPES, 1, OUT)
    b1r = np.ascontiguousarray(b1.reshape(MH, P).T).astype(np.float32)
    b2r = np.ascontiguousarray(b2.reshape(MH, P).T).astype(np.float32)
    R = sum(caps)
    in_maps = []
    for c in range(N_CORES):
        xc = np.zeros((R, LATENT), np.float32)
        off = 0
        for tt in range(TYPES):
            idx = idx_by_type[tt][c]
            xc[off : off + len(idx)] = node_latent[idx]
            off += caps[tt]
        in_maps.append(
            {
                "xt": _tf32(xc.T),
                "w1": _tf32(w1),
                "w2": _tf32(w2),
                "b1r": b1r,
                "b2r": b2r,
                "whp": whp,
                "c1": c1,
                "c2": c2,
                "ocol": np.ones((P, 1), np.float32),
                "orow": np.ones((1, P), np.float32),
            }
        )
    return in_maps


def unpack_outputs(results, caps, idx_by_type, n_rows):
    out = np.empty((n_rows, OUT), np.float32)
    for c in range(N_CORES):
        oc = results[c]["out"]  # [OUT, R]
        off = 0
        for tt in range(TYPES):
            idx = idx_by_type[tt][c]
            out[idx] = oc[:, off : off + len(idx)].T
            off += caps[tt]
    return out


def kernel(node_latent, node_types, w1, b1, w2, b2, ln_gamma, ln_beta, head_w, head_b):
    from concourse.bass_utils import run_bass_kernel_spmd

    node_latent = np.asarray(node_latent, dtype=np.float32)
    node_types = np.asarray(node_types)
    blocks, R, caps, idx_by_type = plan(node_types)
    use_c1 = bool(np.any(np.asarray(ln_beta @ head_w + head_b)))
    nc = build_program(blocks, R, use_c1=use_c1)
    in_maps = prep_inputs(
        node_latent, w1, b1, w2, b2, ln_gamma, ln_beta, head_w, head_b,
        caps, idx_by_type,
    )
    res = run_bass_kernel_spmd(nc, in_maps, core_ids=list(range(N_CORES)))
    return unpack_outputs(res.results, caps, idx_by_type, node_latent.shape[0])


# revision 19
# speedup vs baseline: 1.2278x; 1.0674x over previous
"""Trainium2 Bass kernel for nn_AdaptiveDecoder (shared MLP + hard-routed type heads).

Strategy:
  * Host: sort nodes by type; pad each type's count to a multiple of 8*128 and
    split evenly over 8 cores -> every core sees the SAME static layout of
    type-pure 128-row tiles, so the compiled SPMD program bakes in the
    tile->head mapping and the device does zero routing work.
  * Device: keep activations transposed ([feature, nodes]) the whole way so
    the three matmul stages chain without transposes.  LayerNorm's gamma/beta
    are folded into the head weights on the host; the per-node mean/std terms
    enter via K=1 rank-1 accumulation matmuls and one K=1 broadcast matmul.
  * Matmuls run as float32r (full-rate fp32 path on TRN2 TensorE).
"""

import sys

sys.path.insert(0, "/opt/trn_rl_repo")

from contextlib import ExitStack

import numpy as np

N_CORES = 8
LATENT, HIDDEN, OUT, TYPES = 512, 1024, 256, 3
P = 128
NB = 512  # node columns per block (moving-dim max for 4-byte matmul)
KL = LATENT // P  # 4 k-tiles, stage 1
KH = HIDDEN // P  # 8 k-tiles, stage 2 / head
MH = HIDDEN // P  # 8 m-chunks of hidden
MO = OUT // P  # 2 m-chunks of head output
LN_EPS = 1e-5


def build_program(blocks, R, use_c1=True):
    """blocks: list of (type_idx, col_offset, n_cols); R: node columns per core."""
    import concourse.mybir as mybir
    import concourse.tile as tile
    from concourse import bacc

    dt = mybir.dt
    f32, f32r, bf16 = dt.float32, dt.float32r, dt.bfloat16
    AF = mybir.ActivationFunctionType

    nc = bacc.Bacc("TRN2", target_bir_lowering=False, debug=False, num_devices=N_CORES)

    xt = nc.dram_tensor("xt", [LATENT, R], f32r, kind="ExternalInput").ap()
    w1d = nc.dram_tensor("w1", [LATENT, HIDDEN], f32r, kind="ExternalInput").ap()
    w2d = nc.dram_tensor("w2", [HIDDEN, HIDDEN], f32r, kind="ExternalInput").ap()
    b1d = nc.dram_tensor("b1r", [P, MH], f32, kind="ExternalInput").ap()
    b2d = nc.dram_tensor("b2r", [P, MH], f32, kind="ExternalInput").ap()
    whpd = nc.dram_tensor("whp", [TYPES, HIDDEN, OUT], f32r, kind="ExternalInput").ap()
    c1d = nc.dram_tensor("c1", [TYPES, 1, OUT], f32r, kind="ExternalInput").ap()
    c2d = nc.dram_tensor("c2", [TYPES, 1, OUT], f32r, kind="ExternalInput").ap()
    ocold = nc.dram_tensor("ocol", [P, 1], f32r, kind="ExternalInput").ap()
    orowd = nc.dram_tensor("orow", [1, P], f32r, kind="ExternalInput").ap()
    outd = nc.dram_tensor("out", [OUT, R], f32, kind="ExternalOutput").ap()

    def r(ap):  # operands already declared float32r
        return ap

    with tile.TileContext(nc) as tc, ExitStack() as ctx:
        consts = ctx.enter_context(tc.tile_pool(name="consts", bufs=1))
        xt_pool = ctx.enter_context(tc.tile_pool(name="xt", bufs=2))
        h1_pool = ctx.enter_context(tc.tile_pool(name="h1", bufs=2))
        h2_pool = ctx.enter_context(tc.tile_pool(name="h2", bufs=2))
        sq_pool = ctx.enter_context(tc.tile_pool(name="sq", bufs=1))
        hs_pool = ctx.enter_context(tc.tile_pool(name="hs", bufs=2))
        qs_pool = ctx.enter_context(tc.tile_pool(name="qs", bufs=2))
        rv_pool = ctx.enter_context(tc.tile_pool(name="rv", bufs=1))
        ab_pool = ctx.enter_context(tc.tile_pool(name="ab", bufs=2))
        out_pool = ctx.enter_context(tc.tile_pool(name="outp", bufs=2))
        ps_mlp = ctx.enter_context(tc.tile_pool(name="ps_mlp", bufs=2, space="PSUM"))
        ps_head = ctx.enter_context(tc.tile_pool(name="ps_head", bufs=2, space="PSUM"))
        ps_stat = ctx.enter_context(tc.tile_pool(name="ps_stat", bufs=2, space="PSUM"))
        ps_bc = ctx.enter_context(tc.tile_pool(name="ps_bc", bufs=1, space="PSUM"))

        # --- constants / weights, loaded once ---
        w1_sb = consts.tile([P, KL * HIDDEN], f32r)
        for k in range(KL):
            nc.sync.dma_start(
                out=w1_sb[:, k * HIDDEN : (k + 1) * HIDDEN],
                in_=w1d[k * P : (k + 1) * P, :],
            )
        w2_sb = consts.tile([P, KH * HIDDEN], f32r)
        for k in range(KH):
            nc.sync.dma_start(
                out=w2_sb[:, k * HIDDEN : (k + 1) * HIDDEN],
                in_=w2d[k * P : (k + 1) * P, :],
            )
        whp_sb = consts.tile([P, TYPES * KH * OUT], f32r)
        for t in range(TYPES):
            for k in range(KH):
                nc.sync.dma_start(
                    out=whp_sb[:, (t * KH + k) * OUT : (t * KH + k + 1) * OUT],
                    in_=whpd[t, k * P : (k + 1) * P, :],
                )
        b1_sb = consts.tile([P, MH], f32)
        nc.sync.dma_start(out=b1_sb[:], in_=b1d[:])
        b2_sb = consts.tile([P, MH], f32)
        nc.sync.dma_start(out=b2_sb[:], in_=b2d[:])
        c1_sb = consts.tile([1, TYPES * OUT], f32r)
        c2_sb = consts.tile([1, TYPES * OUT], f32r)
        for t in range(TYPES):
            nc.sync.dma_start(out=c1_sb[:, t * OUT : (t + 1) * OUT], in_=c1d[t])
            nc.sync.dma_start(out=c2_sb[:, t * OUT : (t + 1) * OUT], in_=c2d[t])
        ones_col = consts.tile([P, 1], f32r)  # lhsT for partition-sum reduce
        nc.sync.dma_start(out=ones_col[:], in_=ocold[:])
        ones_col_bf = consts.tile([P, 1], bf16)
        nc.vector.memset(ones_col_bf[:], 1.0)
        ones_row = consts.tile([1, P], f32r)  # lhsT for partition broadcast
        nc.sync.dma_start(out=ones_row[:], in_=orowd[:])
        eps_ap = consts.tile([1, 1], f32)
        nc.vector.memset(eps_ap[:], LN_EPS)

        # --- per-block pipeline (software-pipelined: the LN-dependent PE ops
        # of block b are emitted mid-block b+1 so the PE never waits on the
        # ACT/DVE stats chain and the HAM clock stays warm) ---

        def emit_tail(t, c0, nb, ph_list, negmu, sv, rsig):
            # rank-1 corrections close the head psum accumulation groups
            for mc in range(MO):
                ph = ph_list[mc]
                nc.tensor.matmul(
                    ph[:, :nb],
                    lhsT=c2_sb[:, t * OUT + mc * P : t * OUT + (mc + 1) * P],
                    rhs=negmu[:, :nb],
                    start=False,
                    stop=not use_c1,
                )
                if use_c1:
                    nc.tensor.matmul(
                        ph[:, :nb],
                        lhsT=c1_sb[:, t * OUT + mc * P : t * OUT + (mc + 1) * P],
                        rhs=sv[:, :nb],
                        start=False,
                        stop=True,
                    )
            # broadcast rsig across partitions (K=1 matmul), stash in SBUF
            ps_a = ps_bc.tile([P, NB], f32, tag="bc")
            nc.tensor.matmul(
                ps_a[:, :nb], lhsT=ones_row[:], rhs=rsig[:, :nb],
                start=True, stop=True,
            )
            a_sb = ab_pool.tile([P, NB], f32, tag="a")
            nc.scalar.activation(a_sb[:, :nb], ps_a[:, :nb], AF.Identity)
            out_sb = out_pool.tile([P, MO * NB], f32, tag="out")
            for mc in range(MO):
                nc.vector.tensor_mul(
                    out_sb[:, mc * NB : mc * NB + nb], ph_list[mc][:, :nb],
                    a_sb[:, :nb],
                )
                nc.sync.dma_start(
                    out=outd[mc * P : (mc + 1) * P, c0 : c0 + nb],
                    in_=out_sb[:, mc * NB : mc * NB + nb],
                )

        pending = None
        for t, c0, nb in blocks:
            xt_t = xt_pool.tile([P, KL * NB], f32r, tag="xt")
            for k in range(KL):
                nc.sync.dma_start(
                    out=xt_t[:, k * NB : k * NB + nb],
                    in_=xt[k * P : (k + 1) * P, c0 : c0 + nb],
                )

            # stage 1: h1^T = relu(W1^T x + b1)   [HIDDEN, nb]
            h1_t = h1_pool.tile([P, MH * NB], f32r, tag="h1")
            for m in range(MH):
                ps = ps_mlp.tile([P, NB], f32, tag="ps_mlp")
                for k in range(KL):
                    nc.tensor.matmul(
                        ps[:, :nb],
                        lhsT=w1_sb[:, k * HIDDEN + m * P : k * HIDDEN + (m + 1) * P],
                        rhs=xt_t[:, k * NB : k * NB + nb],
                        start=(k == 0),
                        stop=(k == KL - 1),
                    )
                nc.scalar.activation(
                    h1_t[:, m * NB : m * NB + nb],
                    ps[:, :nb],
                    AF.Relu,
                    bias=b1_sb[:, m : m + 1],
                )

            # deferred LN tail of the previous block slots in here: its PE
            # inputs (negmu/sv/rsig) became ready while stage 1 above ran
            if pending is not None:
                pending()
                pending = None

            # stage 2: h2^T = W2^T h1 + b2; squares ride along per chunk
            h2_t = h2_pool.tile([P, MH * NB], f32r, tag="h2")
            sq_t = sq_pool.tile([P, MH * NB], bf16, tag="sq")
            for m in range(MH):
                ps = ps_mlp.tile([P, NB], f32, tag="ps_mlp")
                for k in range(KH):
                    nc.tensor.matmul(
                        ps[:, :nb],
                        lhsT=w2_sb[:, k * HIDDEN + m * P : k * HIDDEN + (m + 1) * P],
                        rhs=h1_t[:, k * NB : k * NB + nb],
                        start=(k == 0),
                        stop=(k == KH - 1),
                    )
                nc.scalar.activation(
                    h2_t[:, m * NB : m * NB + nb],
                    ps[:, :nb],
                    AF.Identity,
                    bias=b2_sb[:, m : m + 1],
                )
                nc.vector.tensor_mul(
                    sq_t[:, m * NB : m * NB + nb],
                    h2_t[:, m * NB : m * NB + nb].bitcast(f32),
                    h2_t[:, m * NB : m * NB + nb].bitcast(f32),
                )

            # head main matmuls: only need h2, so they keep the PE hot while
            # the stats chain below runs on ACT/DVE
            ph_list = []
            for mc in range(MO):
                ph = ps_head.tile([P, NB], f32, tag="head")
                for k in range(KH):
                    nc.tensor.matmul(
                        ph[:, :nb],
                        lhsT=whp_sb[
                            :,
                            (t * KH + k) * OUT + mc * P : (t * KH + k) * OUT
                            + (mc + 1) * P,
                        ],
                        rhs=h2_t[:, k * NB : k * NB + nb],
                        start=(k == 0),
                        stop=False,
                    )
                ph_list.append(ph)

            # LN stats: pairwise-add tile pairs on DVE, then column sums of
            # the halved sets via ones-matmul (keeps PE work low)
            hs_t = hs_pool.tile([P, (MH // 2) * NB], bf16, tag="hs")
            qs_t = qs_pool.tile([P, (MH // 2) * NB], bf16, tag="qs")
            for k in range(MH // 2):
                nc.vector.tensor_add(
                    hs_t[:, k * NB : k * NB + nb],
                    h2_t[:, 2 * k * NB : 2 * k * NB + nb].bitcast(f32),
                    h2_t[:, (2 * k + 1) * NB : (2 * k + 1) * NB + nb].bitcast(f32),
                )
                nc.vector.tensor_add(
                    qs_t[:, k * NB : k * NB + nb],
                    sq_t[:, 2 * k * NB : 2 * k * NB + nb],
                    sq_t[:, (2 * k + 1) * NB : (2 * k + 1) * NB + nb],
                )
            ps_s = ps_stat.tile([1, NB], f32, tag="stat")
            for k in range(MH // 2):
                nc.tensor.matmul(
                    ps_s[:, :nb],
                    lhsT=ones_col_bf[:],
                    rhs=hs_t[:, k * NB : k * NB + nb],
                    start=(k == 0),
                    stop=(k == MH // 2 - 1),
                )
            ps_q = ps_stat.tile([1, NB], f32, tag="stat")
            for k in range(MH // 2):
                nc.tensor.matmul(
                    ps_q[:, :nb],
                    lhsT=ones_col_bf[:],
                    rhs=qs_t[:, k * NB : k * NB + nb],
                    start=(k == 0),
                    stop=(k == MH // 2 - 1),
                )

            negmu = rv_pool.tile([1, NB], f32r, tag="negmu")
            nc.scalar.activation(
                negmu[:, :nb], ps_s[:, :nb], AF.Identity, scale=-1.0 / HIDDEN
            )
            musq = rv_pool.tile([1, NB], f32, tag="musq")
            nc.scalar.activation(
                musq[:, :nb], ps_s[:, :nb], AF.Square, scale=1.0 / HIDDEN
            )
            varv = rv_pool.tile([1, NB], f32, tag="varv")
            nc.scalar.activation(
                varv[:, :nb], ps_q[:, :nb], AF.Identity, scale=1.0 / HIDDEN
            )
            nc.vector.tensor_sub(varv[:, :nb], varv[:, :nb], musq[:, :nb])
            sv = rv_pool.tile([1, NB], f32r, tag="sv")  # sqrt(var + eps)
            nc.scalar.activation(sv[:, :nb], varv[:, :nb], AF.Sqrt, bias=eps_ap[:])
            rsig = rv_pool.tile([1, NB], f32r, tag="rsig")
            with nc.allow_low_precision(reason="tf32 rsig feeds f32r bcast matmul"):
                nc.vector.reciprocal(rsig[:, :nb], sv[:, :nb].bitcast(f32))

            import functools

            pending = functools.partial(
                emit_tail, t, c0, nb, ph_list, negmu, sv, rsig
            )

        pending()

    nc.compile()
    return nc


def plan(node_types):
    """Host-side layout plan shared by all cores.

    Returns (blocks, R, caps, idx_by_type) where idx_by_type[t][c] is the array
    of original row indices of type t assigned to core c.
    """
    node_types = np.asarray(node_types)
    counts = np.bincount(node_types, minlength=TYPES)
    caps = []  # per-core column capacity for each type (multiple of P)
    idx_by_type = []
    order = np.argsort(node_types, kind="stable")
    starts = np.concatenate([[0], np.cumsum(counts)])
    for tt in range(TYPES):
        tiles = int(-(-counts[tt] // (N_CORES * P)))  # ceil to 128-row tiles/core
        if tiles % 4 == 1:
            # a lone 128-col block runs f32r at 1/4 rate - same cost as 2 cols
            tiles += 1
        cap = tiles * P
        caps.append(cap)
        idx_t = order[starts[tt] : starts[tt + 1]]
        base, rem = divmod(int(counts[tt]), N_CORES)
        parts, o = [], 0
        for c in range(N_CORES):
            n = base + (1 if c < rem else 0)
            parts.append(idx_t[o : o + n])
            o += n
        idx_by_type.append(parts)
    R = sum(caps)
    blocks = []
    off = 0
    for tt in range(TYPES):
        tiles = caps[tt] // P
        j = 0
        while j < tiles:
            nt = min(NB // P, tiles - j)
            blocks.append((tt, off + j * P, nt * P))
            j += nt
        off += caps[tt]
    return blocks, R, caps, idx_by_type


def _tf32(x):
    """Round fp32 to TF32 (10-bit mantissa, round-to-nearest-even)."""
    u = np.ascontiguousarray(x, dtype=np.float32).view(np.uint32).copy()
    lsb = (u >> np.uint32(13)) & np.uint32(1)
    u += np.uint32(0x0FFF) + lsb
    u &= np.uint32(0xFFFFE000)
    return u.view(np.float32)


def prep_inputs(node_latent, w1, b1, w2, b2, ln_gamma, ln_beta, head_w, head_b,
                caps, idx_by_type):
    """Build the 8 per-core input maps."""
    whp = _tf32(ln_gamma[:, None] * head_w)  # [T, H, OUT]
    c1 = _tf32(np.asarray(ln_beta @ head_w + head_b)).reshape(TYPES, 1, OUT)
    c2 = _tf32(np.asarray(ln_gamma @ head_w)).reshape(TYPES, 1, OUT)
    b1r = np.ascontiguousarray(b1.reshape(MH, P).T).astype(np.float32)
    b2r = np.ascontiguousarray(b2.reshape(MH, P).T).astype(np.float32)
    R = sum(caps)
    in_maps = []
    for c in range(N_CORES):
        xc = np.zeros((R, LATENT), np.float32)
        off = 0
        for tt in range(TYPES):
            idx = idx_by_type[tt][c]
            xc[off : off + len(idx)] = node_latent[idx]
            off += caps[tt]
        in_maps.append(
            {
                "xt": _tf32(xc.T),
                "w1": _tf32(w1),
                "w2": _tf32(w2),
                "b1r": b1r,
                "b2r": b2r,
                "whp": whp,
                "c1": c1,
                "c2": c2,
                "ocol": np.ones((P, 1), np.float32),
                "orow": np.ones((1, P), np.float32),
            }
        )
    return in_maps


def unpack_outputs(results, caps, idx_by_type, n_rows):
    out = np.empty((n_rows, OUT), np.float32)
    for c in range(N_CORES):
        oc = results[c]["out"]  # [OUT, R]
        off = 0
        for tt in range(TYPES):
            idx = idx_by_type[tt][c]
            out[idx] = oc[:, off : off + len(idx)].T
            off += caps[tt]
    return out


def kernel(node_latent, node_types, w1, b1, w2, b2, ln_gamma, ln_beta, head_w, head_b):
    from concourse.bass_utils import run_bass_kernel_spmd

    node_latent = np.asarray(node_latent, dtype=np.float32)
    node_types = np.asarray(node_types)
    blocks, R, caps, idx_by_type = plan(node_types)
    use_c1 = bool(np.any(np.asarray(ln_beta @ head_w + head_b)))
    nc = build_program(blocks, R, use_c1=use_c1)
    in_maps = prep_inputs(
        node_latent, w1, b1, w2, b2, ln_gamma, ln_beta, head_w, head_b,
        caps, idx_by_type,
    )
    res = run_bass_kernel_spmd(nc, in_maps, core_ids=list(range(N_CORES)))
    return unpack_outputs(res.results, caps, idx_by_type, node_latent.shape[0])
